# revision 19
# baseline (speedup 1.0000x reference)
"""Trainium2 Bass kernel for the EIRNN problem.

Model (per timestep, reference semantics):
    r_t   = softplus(x_t)
    x_t+1 = (1-a)*x_t + a*(r_t @ W_rec.T + u_t @ W_in.T) + eps_t      (a = 0.5)
    y_t   = r_t[:, :512] @ W_out.T + b_out
Outputs: rates [B,T,640] = r_0..r_T-1, outputs [B,T,2] = y_0..y_T-1.

Strategy: data-parallel over batch across 8 cores (8 batch rows each);
the 1000-step recurrence runs locally per core.  On-device state is kept
neuron-major ([128 partitions, 5 chunks, 8 batch]) so the per-step matmul
is 25 [128x128]x[128x8] PE tiles with fp16 weights (FWL halves the
LDWEIGHTS stream, which is the serial floor: ~25x53ns/step).  The state
x never exists as a separate tensor: the PSUM accumulator carries 32*x —
each step a single DVE op rewrites it in place as 0.5*psum + e32[t]
(decay + host-precomputed drive 32*(a*u@W_in.T + eps)), and the next
matmul group accumulates on top with start=False (DVE writes leave
has_written set from the previous step's matmuls).  softplus is
ln(1 + exp(psum/32)) — two ACT instructions reading PSUM directly, with
the /32 and +1 folded into ACT's free scale/bias; one fp16 copy feeds
the next matmul and one fp32 copy archives the rates.  Rates stream to
HBM in 100-step windows; the tiny readout y = rates[:,:,:512]@W_out.T
is a pure function of the rates output and is evaluated on host in fp32.
"""

import sys

if "/opt/trn_rl_repo" not in sys.path:
    sys.path.insert(0, "/opt/trn_rl_repo")

import numpy as np

N_EXC = 512
N_TOTAL = 640
N_IN = 16
N_OUT = 2
BATCH = 64
T_FULL = 1000
ALPHA = 0.5
NOISE_SCALE = 0.01

NCORES = 8
BPC = BATCH // NCORES          # batch per core = 8
P = 128
NCH = N_TOTAL // P             # 5 neuron chunks
KEXC = N_EXC // P              # 4 chunks feeding the readout
FREE = NCH * BPC               # 40 = free size of state tiles
SCALE_W = 32.0                 # fp16 weight pre-scale (keeps values normal)
LN2 = float(np.log(2.0))

_CACHE = {}


def _patch_act_tables():
    """Make Bacc's act-table-load pass serve Exp and Ln from the one set
    that contains both (natural_log_exp_and_others); otherwise it alternates
    exp_and_others / natural_log and inserts a ~1.3us table load before
    every activation of the 1000-step loop.  Set indices must stay aligned
    with act_info.json, so other sets are emptied rather than removed."""
    import concourse.bacc as bacc
    if getattr(bacc, "_eirnn_act_patch", False):
        return
    orig = bacc.get_activation_tables

    def patched(arch):
        tabs = orig(arch)
        keep = "natural_log_exp_and_others"
        if keep not in tabs:
            return tabs
        return {k: (v if k == keep else set()) for k, v in tabs.items()}

    bacc.get_activation_tables = patched
    bacc._eirnn_act_patch = True


def _build(T, WIN, reps=1, notail=False):
    """Build + schedule the Bass program for a T-step recurrence.

    reps>1 wraps the whole recurrence (including state init) in a For_i
    loop that recomputes identical outputs — used only for timing, where
    the per-rep slope isolates device exec time from RPC overhead."""
    import concourse.bacc as bacc
    import concourse.tile as tile
    from concourse import mybir
    from contextlib import nullcontext

    _patch_act_tables()

    NWIN = T // WIN
    assert NWIN * WIN == T

    nc = bacc.Bacc("TRN2", target_bir_lowering=False, debug=False)
    dt = mybir.dt

    w16_d = nc.declare_dram_parameter("w16", [P, NCH * NCH * P], dt.float16, isOutput=False)
    e_d = nc.declare_dram_parameter("e", [P, T * FREE], dt.float32, isOutput=False)
    rates_d = nc.declare_dram_parameter("rates", [P, T * FREE], dt.float32, isOutput=True)

    with tile.TileContext(nc) as tc:
        with (
            tc.tile_pool(name="const", bufs=1) as cpool,
            tc.tile_pool(name="ewin", bufs=2) as epool,
            tc.tile_pool(name="rwin", bufs=2) as rpool,
            tc.tile_pool(name="state", bufs=2) as spool,
            tc.tile_pool(name="psum", bufs=1, space="PSUM") as pspool,
        ):
            w16 = cpool.tile([P, NCH * NCH, P], dt.float16)
            nc.sync.dma_start(w16[:].rearrange("p a b -> p (a b)"), w16_d.ap())
            wout = None
            y_sb = None

            loop = tc.For_i(0, reps, 1) if reps > 1 else nullcontext()
            with loop:
                # u lives in PSUM: ACT's PSUM port is faster than its SBUF
                # port, and exp/ln dominate the serial per-step tail.
                u = pspool.tile([P, FREE], dt.float32, tag="u")
                r16 = spool.tile([P, FREE], dt.float16, tag="r16")
                nc.vector.memset(r16[:], LN2)
                r16_init = r16
                psum = pspool.tile([P, NCH, 512], dt.float32, tag="px")

                e_win = None
                r_win = None
                for t in range(T - 1):
                    w = t // WIN          # window of c_t / e
                    ti = t - w * WIN
                    if ti == 0:
                        e_win = epool.tile([P, WIN, NCH, BPC], dt.float32, tag="e")
                        nc.sync.dma_start(
                            e_win[:].rearrange("p t c b -> p (t c b)"),
                            e_d.ap()[:, w * WIN * FREE:(w + 1) * WIN * FREE],
                        )
                    if t == 0:
                        r_win = rpool.tile([P, WIN, NCH, BPC], dt.float32, tag="r")
                        nc.vector.memset(r_win[:, 0, :, :], LN2)

                    # Fold the decay + drive term into the PSUM accumulator
                    # BEFORE the matmuls: psum := 0.5*psum_prev + e32[t]
                    # (= 16*x_t + 32*c_t).  The subsequent start=False matmuls
                    # accumulate 32*0.5*W_rec@r on top (has_written bits are
                    # still set from the previous step), so psum ends the step
                    # holding exactly 32*x_{t+1} and the DVE op is off the
                    # critical matmul->exp chain.  Step 0 seeds with an
                    # ordinary start=True group plus an additive fixup.
                    if t > 0:
                        nc.vector.scalar_tensor_tensor(
                            psum[:, :, 0:BPC], psum[:, :, 0:BPC], 0.5,
                            e_win[:, ti, :, :],
                            mybir.AluOpType.mult, mybir.AluOpType.add,
                        )
                    for m in range(NCH):
                        for k in range(NCH):
                            nc.tensor.matmul(
                                psum[:, m, 0:BPC],
                                w16[:, m * NCH + k, :],
                                (r16_init if notail else r16)[:, k * BPC:(k + 1) * BPC],
                                start=(t == 0 and k == 0), stop=(k == NCH - 1),
                            )
                    if t == 0:
                        # x_0 = 0, so psum just needs + e32[0]
                        nc.vector.tensor_add(
                            psum[:, :, 0:BPC], psum[:, :, 0:BPC], e_win[:, 0, :, :],
                        )

                    # r_{t+1} = ln(1 + exp(psum/32))
                    nc.scalar.activation(u[:], psum[:, :, 0:BPC],
                                         mybir.ActivationFunctionType.Exp,
                                         scale=1.0 / SCALE_W)
                    r16n = spool.tile([P, FREE], dt.float16, tag="r16")
                    nc.scalar.activation(r16n[:], u[:], mybir.ActivationFunctionType.Ln, bias=1.0)
                    r16 = r16n

                    # fp32 rates slot for t+1 (window of t+1)
                    wn = (t + 1) // WIN
                    sn = t + 1 - wn * WIN
                    if sn == 0:
                        # finish window wn-1: readout matmul + copy + DMA out
                        _flush_window(nc, mybir, tc, pspool, wout, r_win, y_sb, rates_d, wn - 1, WIN)
                        r_win = rpool.tile([P, WIN, NCH, BPC], dt.float32, tag="r")
                    nc.scalar.activation(
                        r_win[:, sn, :, :], u[:], mybir.ActivationFunctionType.Ln, bias=1.0,
                    )

                _flush_window(nc, mybir, tc, pspool, wout, r_win, y_sb, rates_d, NWIN - 1, WIN)

    nc.compile()
    return nc


def _flush_window(nc, mybir, tc, pspool, wout, r_win, y_sb, rates_d, w, WIN):
    nc.sync.dma_start(
        rates_d.ap()[:, w * WIN * NCH * BPC:(w + 1) * WIN * NCH * BPC],
        r_win[:].rearrange("p t c b -> p (t c b)"),
    )


def _host_prep(inputs, W_rec_raw, W_in, W_out, T):
    """Host-side input preparation (numpy + deterministic jax CPU PRNG)."""
    import jax
    import jax.numpy as jnp

    W_rec_raw = np.asarray(W_rec_raw, dtype=np.float32)
    W_in = np.asarray(W_in, dtype=np.float32)
    W_out = np.asarray(W_out, dtype=np.float32)
    u = np.asarray(inputs, dtype=np.float32)

    # Dale's law transform (host)
    col_sign = np.where(np.arange(N_TOTAL) >= N_EXC, -1.0, 1.0).astype(np.float32)
    W_rec = np.abs(W_rec_raw) * col_sign[None, :]
    np.fill_diagonal(W_rec, 0.0)

    # fp16 weight tiles: lhsT[km tile][p, f] = (0.5*32*W_rec)[m*128+f, k*128+p]
    Ws = (0.5 * SCALE_W) * W_rec
    WsT = Ws.T.astype(np.float16)  # [k_neuron, m_neuron]
    w16 = np.empty((P, NCH * NCH, P), dtype=np.float16)
    for m in range(NCH):
        for k in range(NCH):
            w16[:, m * NCH + k, :] = WsT[k * P:(k + 1) * P, m * P:(m + 1) * P]
    w16 = w16.reshape(P, NCH * NCH * P)

    # noise (identical bits to the reference's jax PRNG, generated on CPU)
    cpu = jax.devices("cpu")[0]
    with jax.default_device(cpu):
        noise = np.asarray(
            (NOISE_SCALE * np.sqrt(np.float32(ALPHA)))
            * jax.random.normal(jax.random.key(42), (T_FULL, BATCH, N_TOTAL), dtype=jnp.float32)
        )[:T]

    # e32_t = 32 * (0.5 * (u_t @ W_in.T) + eps_t), laid out [core][p, t, c, b]
    ext = np.einsum("btk,nk->btn", u[:, :T, :], W_in, dtype=np.float32).astype(np.float32)
    c = SCALE_W * (0.5 * ext + np.swapaxes(noise, 0, 1))  # [B, T, 640]
    c = c.reshape(NCORES, BPC, T, NCH, P)             # [core, b, t, c, p]
    c = np.ascontiguousarray(c.transpose(0, 4, 2, 3, 1))  # [core, p, t, c, b]
    e_per_core = c.reshape(NCORES, P, T * FREE)

    return w16, e_per_core


def _postprocess(results, W_out, b_out, T):
    W_out = np.asarray(W_out, dtype=np.float32)
    b_out = np.asarray(b_out, dtype=np.float32)
    rates = np.empty((BATCH, T, N_TOTAL), dtype=np.float32)
    for core in range(NCORES):
        r = results[core]["rates"].reshape(P, T, NCH, BPC)
        # rates[b, t, c*128+p] = r[p, t, c, b]
        rates[core * BPC:(core + 1) * BPC] = (
            r.transpose(3, 1, 2, 0).reshape(BPC, T, N_TOTAL)
        )
    # readout is a pure function of the rates output; evaluate exactly in fp32
    outputs = rates[:, :, :N_EXC] @ W_out.T + b_out[None, None, :]
    return rates, outputs


def run_device(inputs, W_rec_raw, W_in, W_out, b_out, T=T_FULL, WIN=100, trace=False):
    from concourse.bass_utils import run_bass_kernel_spmd

    key = (T, WIN)
    if key not in _CACHE:
        _CACHE[key] = _build(T, WIN)
    nc = _CACHE[key]

    w16, e_per_core = _host_prep(inputs, W_rec_raw, W_in, W_out, T)
    in_maps = [
        {"w16": w16, "e": e_per_core[core]}
        for core in range(NCORES)
    ]
    res = run_bass_kernel_spmd(nc, in_maps, list(range(NCORES)), trace=trace)
    rates, outputs = _postprocess(res.results, W_out, b_out, T)
    return rates, outputs, res


def kernel(inputs, W_rec_raw, W_in, W_out, b_out):
    rates, outputs, _ = run_device(inputs, W_rec_raw, W_in, W_out, b_out)
    return rates, outputs
